# revision 1
# baseline (speedup 1.0000x reference)
"""Multi-head self-attention (B=2, S=2048, E=1024, H=16, D=64) on 8 NeuronCores.

Sharding: core c -> (batch b = c // 4, head group g = c % 4).  Each core
computes Q/K/V projections for its 4 heads (column-parallel), attention, and
a partial output projection (row-parallel); the host sums the 4 partials per
batch.  All device activations live in "transposed space" (feature on the
partition dim) so every matmul contracts along partitions with no on-device
transposes:

  Q^T = Wq_g^T @ X^T          [256, 2048]  (e-chunk accumulated, + bq)
  K^T = Wk_g^T @ X^T          [256, 2048]
  V   = X @ Wv_g              [2048, 256]  (natural; ones column appended)
  S^T = K_h @ Q_h^T / 8       [2048, 2048] per head (computed tile-wise)
  P^T = exp(S^T)              (softmax without max-subtraction: scores ~N(0,1))
  O'^T = [V_h | 1]^T @ P^T    [65, q]  (row 64 = softmax denominators)
  O^T  = O'[0:64] / O'[64]    (DVE reciprocal + GpSimd partition broadcast)
  Y^T  = Wo_g^T @ O^T         [1024, 2048] partial, host-summed per batch

bv and bo are folded on the host (exact: softmax rows sum to 1, so
attn(V + bv) = attn(V) + bv, and the output projection is linear).
"""

from contextlib import ExitStack

import numpy as np

import concourse.bass as bass
import concourse.tile as tile
from concourse import bacc, mybir
from concourse.bass_utils import run_bass_kernel_spmd

B, S, E, H, D = 2, 2048, 1024, 16, 64
NCORES = 8
GH = 4            # heads per core
DC = GH * D       # head-dim columns per core (256)
EC = E // 128     # 8 e-chunks
KC = S // 128     # 16 k-chunks
F32 = mybir.dt.float32
MM_DT = mybir.dt.float16    # full-speed 16-bit matmul path (10-bit mantissa)
EXP_FUNC = mybir.ActivationFunctionType.Exp
SCALE = 1.0 / np.sqrt(np.float32(D))


def _mm(ap):
    return ap


def round_f32r(a):
    # Host-side conversion to the matmul dtype (RNE)
    if MM_DT == mybir.dt.float16:
        return np.ascontiguousarray(a, np.float32).astype(np.float16)
    if MM_DT == mybir.dt.bfloat16:
        import ml_dtypes
        return np.ascontiguousarray(a, np.float32).astype(ml_dtypes.bfloat16)
    if MM_DT == mybir.dt.float32r:
        u = np.ascontiguousarray(a, np.float32).view(np.uint32)
        u = ((u.astype(np.uint64) + 0x800) & 0xFFFFF000).astype(np.uint32)
        return u.view(np.float32)
    return np.ascontiguousarray(a, np.float32)


DEBUG_DUMPS = False


def _emit(nc, tc, ctx, xT, wq, wk, wv, wo, bq, bk, yT, dbg=None):
    sb_big = ctx.enter_context(tc.tile_pool(name="sb_big", bufs=1))
    sb_p = ctx.enter_context(tc.tile_pool(name="sb_p", bufs=28))
    sb_norm = ctx.enter_context(tc.tile_pool(name="sb_norm", bufs=4))
    sb_y = ctx.enter_context(tc.tile_pool(name="sb_y", bufs=2))
    ps_big = ctx.enter_context(tc.tile_pool(name="ps_big", bufs=2, space="PSUM"))
    ps_acc = ctx.enter_context(tc.tile_pool(name="ps_acc", bufs=2, space="PSUM"))

    xT_t = sb_big.tile([128, EC, S], MM_DT)
    wq_t = sb_big.tile([128, EC, DC], MM_DT)
    wk_t = sb_big.tile([128, EC, DC], MM_DT)
    wv_t = sb_big.tile([128, EC, DC], MM_DT)
    wo_t = sb_big.tile([128, 2, E], MM_DT)
    bqk_t = sb_big.tile([1, 2, DC], MM_DT)
    ones_t = sb_big.tile([1, 512], MM_DT)
    qT_t = sb_big.tile([128, 2, S], MM_DT)
    kT_t = sb_big.tile([128, 2, S], MM_DT)
    v_t = sb_big.tile([128, KC, GH, D + 1], MM_DT)
    o_t = sb_big.tile([128, 2, S], MM_DT)

    # Inputs are host-permuted to the exact SBUF layouts, so every load is a
    # dense per-partition-contiguous copy (cheap descriptors); issues are
    # spread across engine queues to parallelize DMA setup.
    nc.scalar.dma_start(out=wq_t[:, :, :],
                        in_=wq.rearrange("p (c d) -> p c d", c=EC))
    nc.scalar.dma_start(out=wk_t[:, :, :],
                        in_=wk.rearrange("p (c d) -> p c d", c=EC))
    for ec in range(EC):
        eng = nc.sync if ec % 2 == 0 else nc.gpsimd
        eng.dma_start(out=xT_t[:, ec, :], in_=xT[:, ec * S:(ec + 1) * S])
    nc.sync.dma_start(out=wv_t[:, :, :],
                        in_=wv.rearrange("p (c d) -> p c d", c=EC))
    nc.gpsimd.dma_start(out=wo_t[:, :, :],
                        in_=wo.rearrange("p (c e) -> p c e", c=2))
    nc.gpsimd.dma_start(out=bqk_t[:, 0, :], in_=bq[None, :])
    nc.gpsimd.dma_start(out=bqk_t[:, 1, :], in_=bk[None, :])
    nc.vector.memset(ones_t[:, :], 1.0)
    for kc in range(KC):
        nc.vector.memset(v_t[:, kc, :, D:D + 1], 1.0)


    def qk_part(dc, proj, sc, half, state={}):
        # psum[d, s] += W[e, d].T @ X^T[e, s]   (+ bias via K=1 matmul),
        # emitted in two halves so filler bursts stay small
        w_t, dst = ((wq_t, qT_t), (wk_t, kT_t))[proj]
        if half == 0:
            state[(dc, proj, sc)] = ps_big.tile(
                [128, 512], F32, tag="big", name="ps_qk")
        ps = state[(dc, proj, sc)]
        ecs = range(EC // 2) if half == 0 else range(EC // 2, EC)
        for ec in ecs:
            nc.tensor.matmul(
                ps[:, :],
                lhsT=w_t[:, ec, dc * 128:(dc + 1) * 128],
                rhs=xT_t[:, ec, sc * 512:(sc + 1) * 512],
                start=(ec == 0), stop=False)
        if half == 1:
            nc.tensor.matmul(
                ps[:, :],
                lhsT=bqk_t[:, proj, dc * 128:(dc + 1) * 128],
                rhs=ones_t[:, :],
                start=False, stop=True)
            nc.vector.tensor_copy(
                out=dst[:, dc, sc * 512:(sc + 1) * 512], in_=ps[:, :])
            del state[(dc, proj, sc)]

    def qk_group(dc, proj, sc):
        qk_part(dc, proj, sc, 0)
        qk_part(dc, proj, sc, 1)

    def v_proj():
        # psum[s, d] += X^T[e, s].T @ Wv[e, d]
        for kc in range(KC):
            ps = ps_acc.tile([128, 512], F32, tag="acc", bufs=4, name="ps_v")
            for ec in range(EC):
                nc.tensor.matmul(
                    ps[:, 0:DC],
                    lhsT=xT_t[:, ec, kc * 128:(kc + 1) * 128],
                    rhs=wv_t[:, ec, :],
                    start=(ec == 0), stop=(ec == EC - 1))
            nc.vector.tensor_copy(
                out=v_t[:, kc, :, 0:D],
                in_=ps[:, 0:DC].rearrange("p (h d) -> p h d", h=GH))

    def attention_scores(qc, hc, kcs=None):
        # Head pair (2*hc, 2*hc+1): head hp=0 on SBUF partitions 0-63, hp=1
        # on 64-127, so the two scores matmuls run as independent 64x128 PE
        # tiles and one ACTIVATE covers both heads' exp.
        pTs = []
        for kc in (kcs if kcs is not None else range(KC)):
            sco = ps_big.tile([128, 2, 512], F32, tag="big", name="sco")
            for hp in range(2):
                po = hp * 64
                nc.tensor.matmul(
                    sco[:, hp, :],
                    lhsT=kT_t[po:po + 64, hc, kc * 128:(kc + 1) * 128],
                    rhs=qT_t[po:po + 64, hc, qc * 512:(qc + 1) * 512],
                    start=True, stop=True)
            pT = sb_p.tile([128, 2, 512], MM_DT)
            nc.scalar.activation(
                out=pT[:, :, :], in_=sco[:, :, :], func=EXP_FUNC,
                scale=float(SCALE))
            pTs.append(pT)
        return pTs

    def pv_alloc():
        return [ps_acc.tile([128, 512], F32, tag="acc", bufs=4, name=f"acc{j}")
                for j in range(2)]

    def pv_kc(accs, hc, pTs, kc):
        for hp in range(2):
            h = 2 * hc + hp
            nc.tensor.matmul(
                accs[hp][0:D + 1, :],
                lhsT=v_t[:, kc, h, :],
                rhs=pTs[kc][:, hp, :],
                start=(kc == 0), stop=(kc == KC - 1))

    def attention_norm(qc, hc, accs):
        for hp in range(2):
            po = hp * 64
            rs = sb_norm.tile([1, 512], F32, tag="rs")
            nc.vector.tensor_copy(out=rs[:, :], in_=accs[hp][D:D + 1, :])
            inv_r = sb_norm.tile([1, 512], F32, tag="inv")
            nc.vector.reciprocal_approx_fast(out=inv_r[:, :], in_=rs[:, :])
            brd = sb_norm.tile([64, 512], F32, tag="brd")
            nc.gpsimd.partition_broadcast(brd[:, :], inv_r[:, :])
            nc.vector.tensor_mul(
                o_t[po:po + 64, hc, qc * 512:(qc + 1) * 512],
                accs[hp][0:D, :],
                brd[:, :])

    def attention_pv(qc, hc, pTs):
        accs = pv_alloc()
        for kc in range(KC):
            pv_kc(accs, hc, pTs, kc)
        attention_norm(qc, hc, accs)

    def y_group(qc, ec, tag="acc", bufs=4, copy_eng=None):
        # psum[e, s] += Wo[c, e].T @ O^T[c, s] for chunk (ec, qc)
        yp = ps_acc.tile([128, 512], F32, tag=tag, bufs=bufs, name="yp")
        for cc in range(2):
            nc.tensor.matmul(
                yp[:, :],
                lhsT=wo_t[:, cc, ec * 128:(ec + 1) * 128],
                rhs=o_t[:, cc, qc * 512:(qc + 1) * 512],
                start=(cc == 0), stop=(cc == 1))
        ys = sb_y.tile([128, 512], F32)
        if copy_eng == "scalar":
            nc.scalar.copy(out=ys[:, :], in_=yp[:, :])
        else:
            nc.vector.tensor_copy(out=ys[:, :], in_=yp[:, :])
        nc.sync.dma_start(
            out=yT[ec * 128:(ec + 1) * 128, qc * 512:(qc + 1) * 512],
            in_=ys[:, :])

    def y_proj(qc):
        for ec in range(EC):
            y_group(qc, ec)

    # Software-pipelined emission (= Tile priority order).  The exp stream
    # drives the schedule: each attention block emits scores+exp for (qc, hc)
    # at top priority while the PREVIOUS block's PV matmuls and filler work
    # (remaining projections, output-projection chunks) weave in at kc
    # granularity, so ScalarE never starves.
    blocks = [(0, 0), (1, 0), (2, 0), (3, 0), (0, 1), (1, 1), (2, 1), (3, 1)]

    def qk1(proj, sc, half):
        return lambda: qk_part(1, proj, sc, half)

    def qk0(proj, sc, half):
        return lambda: qk_part(0, proj, sc, half)

    # filler generators keyed by block index: list of (after_kc, fn)
    fillers = {
        1: [(3, qk0(0, 2, 0)), (5, qk0(0, 2, 1)),
            (11, qk0(0, 3, 0)), (13, qk0(0, 3, 1))],
        2: [(1, qk1(0, 0, 0)), (3, qk1(0, 0, 1)),
            (5, qk1(1, 0, 0)), (7, qk1(1, 0, 1)),
            (9, qk1(0, 1, 0)), (11, qk1(0, 1, 1)),
            (13, qk1(1, 1, 0)), (15, qk1(1, 1, 1))],
        3: [(1, qk1(0, 2, 0)), (3, qk1(0, 2, 1)),
            (5, qk1(1, 2, 0)), (7, qk1(1, 2, 1)),
            (9, qk1(0, 3, 0)), (11, qk1(0, 3, 1)),
            (13, qk1(1, 3, 0)), (15, qk1(1, 3, 1))],
        6: [(2 * i + 1, (lambda e: lambda: y_group(0, e))(i)) for i in range(EC)],
        7: [(2 * i + 1, (lambda e: lambda: y_group(1, e))(i)) for i in range(EC)],
    }

    # staged startup: emit first-block scores as soon as each kT s-chunk's
    # projection is emitted, so the exp stream starts ~20us earlier
    qk_group(0, 0, 0)
    qk_group(0, 1, 0)
    pts_prev = attention_scores(0, 0, range(0, 4))
    qk_group(0, 1, 1)
    pts_prev += attention_scores(0, 0, range(4, 8))
    qk_group(0, 1, 2)
    pts_prev += attention_scores(0, 0, range(8, 12))
    qk_group(0, 1, 3)
    pts_prev += attention_scores(0, 0, range(12, 16))
    qk_group(0, 0, 1)
    v_proj()
    prev_block = (0, 0)
    for bi in range(1, len(blocks)):
        qc, hc = blocks[bi]
        pqc, phc = prev_block
        accs = pv_alloc()
        pts_cur = []
        fl = dict((k, f) for k, f in fillers.get(bi, []))
        for kc in range(KC):
            pts_cur += attention_scores(qc, hc, [kc])
            pv_kc(accs, phc, pts_prev, kc)
            if kc in fl:
                fl[kc]()
        attention_norm(pqc, phc, accs)
        pts_prev = pts_cur
        prev_block = (qc, hc)
    # final block: PV + norm + remaining output projection
    accs = pv_alloc()
    for kc in range(KC):
        pv_kc(accs, prev_block[1], pts_prev, kc)
        if kc % 2 == 1:
            y_group(2, kc // 2)
    attention_norm(prev_block[0], prev_block[1], accs)
    for ec in range(EC):
        y_group(3, ec, copy_eng="scalar" if ec % 2 else None)

    if dbg is not None:
        for name, t in (("qT", qT_t), ("kT", kT_t), ("o", o_t)):
            f = sb_big.tile([128, 2, S], F32, name=f"dump_{name}")
            nc.vector.tensor_copy(out=f[:, :, :], in_=t[:, :, :])
            nc.sync.dma_start(out=dbg[name], in_=f.rearrange("p a b -> p (a b)"))
        fv = sb_big.tile([128, KC, GH, D + 1], F32, name="dump_v")
        nc.vector.tensor_copy(out=fv[:, :, :, :], in_=v_t[:, :, :, :])
        nc.sync.dma_start(out=dbg["v"], in_=fv.rearrange("p a b c -> p (a b c)"))


_cached_nc = None


def _build():
    nc = bacc.Bacc(trn_type="TRN2", target_bir_lowering=False)
    xT = nc.dram_tensor("xT", [128, EC * S], MM_DT, kind="ExternalInput").ap()
    wq = nc.dram_tensor("wq", [128, EC * DC], MM_DT, kind="ExternalInput").ap()
    wk = nc.dram_tensor("wk", [128, EC * DC], MM_DT, kind="ExternalInput").ap()
    wv = nc.dram_tensor("wv", [128, EC * DC], MM_DT, kind="ExternalInput").ap()
    wo = nc.dram_tensor("wo", [128, 2 * E], MM_DT, kind="ExternalInput").ap()
    bq = nc.dram_tensor("bq", [DC], MM_DT, kind="ExternalInput").ap()
    bk = nc.dram_tensor("bk", [DC], MM_DT, kind="ExternalInput").ap()
    yT = nc.dram_tensor("yT", [E, S], F32, kind="ExternalOutput").ap()
    dbg = None
    if DEBUG_DUMPS:
        dbg = {
            "qT": nc.dram_tensor("dbg_qT", [128, 2 * S], F32, kind="ExternalOutput").ap(),
            "kT": nc.dram_tensor("dbg_kT", [128, 2 * S], F32, kind="ExternalOutput").ap(),
            "o": nc.dram_tensor("dbg_o", [128, 2 * S], F32, kind="ExternalOutput").ap(),
            "v": nc.dram_tensor("dbg_v", [128, KC * GH * (D + 1)], F32, kind="ExternalOutput").ap(),
        }
    with tile.TileContext(nc) as tc:
        with ExitStack() as ctx:
            _emit(nc, tc, ctx, xT, wq, wk, wv, wo, bq, bk, yT, dbg)
    nc.compile()
    return nc


def get_nc():
    global _cached_nc
    if _cached_nc is None:
        _cached_nc = _build()
    return _cached_nc


def make_in_maps(inputs, wq, bq, wk, bk, wv, wo):
    in_maps = []
    for c in range(NCORES):
        b, g = divmod(c, GH)
        sl = slice(g * DC, (g + 1) * DC)
        def perm(a):
            # [C*128, N] -> [128, C*N] with SBUF chunk-major free dim
            cN = a.shape[0] // 128
            return np.ascontiguousarray(
                a.reshape(cN, 128, a.shape[1]).transpose(1, 0, 2).reshape(
                    128, cN * a.shape[1]))

        in_maps.append({
            "xT": round_f32r(perm(np.ascontiguousarray(inputs[b].T))),
            "wq": round_f32r(perm(wq[:, sl])),
            "wk": round_f32r(perm(wk[:, sl])),
            "wv": round_f32r(perm(wv[:, sl])),
            "wo": round_f32r(perm(wo[sl, :])),
            "bq": round_f32r(bq[sl]),
            "bk": round_f32r(bk[sl]),
        })
    return in_maps


def combine(results, wv_full, bv, wo_full, bo):
    y = np.zeros((B, S, E), np.float32)
    for c in range(NCORES):
        y[c // GH] += results[c]["yT"].T
    y += bv @ wo_full + bo
    return y


def kernel(inputs, wq, bq, wk, bk, wv, bv, wo, bo, _run_kwargs=None):
    inputs = np.asarray(inputs, np.float32)
    wq, bq = np.asarray(wq, np.float32), np.asarray(bq, np.float32)
    wk, bk = np.asarray(wk, np.float32), np.asarray(bk, np.float32)
    wv, bv = np.asarray(wv, np.float32), np.asarray(bv, np.float32)
    wo, bo = np.asarray(wo, np.float32), np.asarray(bo, np.float32)

    nc = get_nc()
    in_maps = make_in_maps(inputs, wq, bq, wk, bk, wv, wo)
    res = run_bass_kernel_spmd(nc, in_maps, list(range(NCORES)),
                               **(_run_kwargs or {}))
    y = combine(res.results, wv, bv, wo, bo)
    if _run_kwargs:
        kernel.last_result = res
    return y



# revision 5
# speedup vs baseline: 1.0588x; 1.0588x over previous
"""Multi-head self-attention (B=2, S=2048, E=1024, H=16, D=64) on 8 NeuronCores.

Sharding: core c -> (batch b = c // 4, head group g = c % 4).  Each core
computes Q/K/V projections for its 4 heads (column-parallel), attention, and
a partial output projection (row-parallel); the host sums the 4 partials per
batch.  All device activations live in "transposed space" (feature on the
partition dim) so every matmul contracts along partitions with no on-device
transposes:

  Q^T = Wq_g^T @ X^T          [256, 2048]  (e-chunk accumulated; bias via DVE)
  K^T = Wk_g^T @ X^T          [256, 2048]
  V   = X @ Wv_g              [2048, 256]  (natural; ones column appended)
  S^T = K_h @ Q_h^T / 8       [2048, 2048] per head (computed tile-wise)
  P^T = exp(S^T)              (softmax without max-subtraction: scores ~N(0,1))
  O'^T = [V_h | 1]^T @ P^T    [65, q]  (row 64 = softmax denominators)
  O^T  = O'[0:64] / O'[64]    (DVE reciprocal + GpSimd partition broadcast)
  Y^T  = Wo_g^T @ O^T         [1024, 2048] partial, host-summed per batch

bv and bo are folded on the host (exact: softmax rows sum to 1, so
attn(V + bv) = attn(V) + bv, and the output projection is linear).

Schedule: the exp stream (ScalarE) and the matmul stream (PE) are both near
their engine floors (~128us and ~140us), so the emission order software-
pipelines them: per k-chunk the PE emits the next scores pair + the previous
block's PV accumulation + one "filler" (projection / output chunk) sized to
keep PE just under the ACT rate.  Warmup matmuls + a dummy exp run during the
initial DMA so the PE starts HAM-warm and the exp table set is preloaded.
"""

from contextlib import ExitStack

import numpy as np

import concourse.bass as bass
import concourse.tile as tile
from concourse import bacc, mybir
from concourse.bass_utils import run_bass_kernel_spmd

B, S, E, H, D = 2, 2048, 1024, 16, 64
NCORES = 8
GH = 4            # heads per core
DC = GH * D       # head-dim columns per core (256)
EC = E // 128     # 8 e-chunks
KC = S // 128     # 16 k-chunks
F32 = mybir.dt.float32
MM_DT = mybir.dt.float16    # full-speed 16-bit matmul path (10-bit mantissa)
EXP_FUNC = mybir.ActivationFunctionType.Exp
ADD = mybir.AluOpType.add
SCALE = 1.0 / np.sqrt(np.float32(D))


def round_f32r(a):
    # Host-side conversion to the matmul dtype (RNE)
    if MM_DT == mybir.dt.float16:
        return np.ascontiguousarray(a, np.float32).astype(np.float16)
    if MM_DT == mybir.dt.bfloat16:
        import ml_dtypes
        return np.ascontiguousarray(a, np.float32).astype(ml_dtypes.bfloat16)
    return np.ascontiguousarray(a, np.float32)


DEBUG_DUMPS = False


def _emit(nc, tc, ctx, xT, wq, wk, wv, wo, bqk, yT, dbg=None):
    sb_big = ctx.enter_context(tc.tile_pool(name="sb_big", bufs=1))
    sb_p = ctx.enter_context(tc.tile_pool(name="sb_p", bufs=28))
    sb_norm = ctx.enter_context(tc.tile_pool(name="sb_norm", bufs=4))
    sb_y = ctx.enter_context(tc.tile_pool(name="sb_y", bufs=2))
    ps_sco = ctx.enter_context(tc.tile_pool(name="ps_sco", bufs=2, space="PSUM"))
    ps_acc = ctx.enter_context(tc.tile_pool(name="ps_acc", bufs=4, space="PSUM"))

    xT_t = sb_big.tile([128, EC, S], MM_DT)
    wq_t = sb_big.tile([128, EC, DC], MM_DT)
    wk_t = sb_big.tile([128, EC, DC], MM_DT)
    wv_t = sb_big.tile([128, EC, DC], MM_DT)
    wo_t = sb_big.tile([128, 2, E], MM_DT)
    bqk_t = sb_big.tile([128, 4], F32)
    qT_t = sb_big.tile([128, 2, S], MM_DT)
    kT_t = sb_big.tile([128, 2, S], MM_DT)
    v_t = sb_big.tile([128, KC, GH, D + 1], MM_DT)
    o_t = sb_big.tile([128, 2, S], MM_DT)
    warm_t = sb_big.tile([128, 512], MM_DT)
    warm_o = sb_big.tile([128, 512], MM_DT)

    # --- warmup: PE busy + exp table preload while input DMAs run -------
    nc.vector.memset(warm_t[:, :], 0.125)
    nc.scalar.activation(out=warm_o[:, :], in_=warm_t[:, :], func=EXP_FUNC,
                         scale=float(SCALE))
    warm_ps = ps_acc.tile([128, 512], F32, tag="acc", name="warm")
    for i in range(18):
        nc.tensor.matmul(warm_ps[:, :], lhsT=warm_t[:, 0:128],
                         rhs=warm_t[:, :], start=True, stop=True)
    for kc in range(KC):
        nc.vector.memset(v_t[:, kc, :, D:D + 1], 1.0)

    # --- input DMAs: ordered so first-needed tiles land first -----------
    # sync queue: wq(dc0), x halves (ec 0-3) per sc, wv, wo(c0)
    # gpsimd queue: bias, wk(dc0), x halves (ec 4-7) per sc, wq/wk(dc1), wo(c1)
    wq_r = wq.rearrange("p (c d) -> p c d", c=EC)
    wk_r = wk.rearrange("p (c d) -> p c d", c=EC)
    x_r = xT.rearrange("p (c s) -> p c s", c=EC)
    wo_r = wo.rearrange("p (c e) -> p c e", c=2)
    nc.sync.dma_start(out=wq_t[:, :, 0:128], in_=wq_r[:, :, 0:128])
    nc.gpsimd.dma_start(out=bqk_t[:, :], in_=bqk)
    nc.gpsimd.dma_start(out=wk_t[:, :, 0:128], in_=wk_r[:, :, 0:128])
    for sc in range(4):
        sl = slice(sc * 512, (sc + 1) * 512)
        nc.sync.dma_start(out=xT_t[:, 0:4, sl], in_=x_r[:, 0:4, sl])
        nc.gpsimd.dma_start(out=xT_t[:, 4:8, sl], in_=x_r[:, 4:8, sl])
    nc.sync.dma_start(out=wv_t[:, :, :], in_=wv.rearrange(
        "p (c d) -> p c d", c=EC))
    nc.gpsimd.dma_start(out=wq_t[:, :, 128:256], in_=wq_r[:, :, 128:256])
    nc.gpsimd.dma_start(out=wk_t[:, :, 128:256], in_=wk_r[:, :, 128:256])
    nc.sync.dma_start(out=wo_t[:, 0, :], in_=wo_r[:, 0, :])
    nc.gpsimd.dma_start(out=wo_t[:, 1, :], in_=wo_r[:, 1, :])

    def qk_part(dc, proj, sc, half, state={}):
        # psum[d, s] += W[e, d].T @ X^T[e, s], two halves so bursts stay
        # small; bias folded into the DVE evacuation copy.
        w_t, dst = ((wq_t, qT_t), (wk_t, kT_t))[proj]
        if half == 0:
            state[(dc, proj, sc)] = ps_acc.tile(
                [128, 512], F32, tag="acc", name="ps_qk")
        ps = state[(dc, proj, sc)]
        ecs = range(EC // 2) if half == 0 else range(EC // 2, EC)
        for ec in ecs:
            nc.tensor.matmul(
                ps[:, :],
                lhsT=w_t[:, ec, dc * 128:(dc + 1) * 128],
                rhs=xT_t[:, ec, sc * 512:(sc + 1) * 512],
                start=(ec == 0), stop=(ec == EC - 1))
        if half == 1:
            nc.vector.tensor_scalar(
                out=dst[:, dc, sc * 512:(sc + 1) * 512], in0=ps[:, :],
                scalar1=bqk_t[:, 2 * proj + dc:2 * proj + dc + 1],
                scalar2=None, op0=ADD)
            del state[(dc, proj, sc)]

    def v_part(kc, half):
        # psum[s, d] += X^T[e, s].T @ Wv[e, d]
        if half == 0:
            v_part.ps = ps_acc.tile([128, 512], F32, tag="acc", name="ps_v")
        ps = v_part.ps
        ecs = range(EC // 2) if half == 0 else range(EC // 2, EC)
        for ec in ecs:
            nc.tensor.matmul(
                ps[:, 0:DC],
                lhsT=xT_t[:, ec, kc * 128:(kc + 1) * 128],
                rhs=wv_t[:, ec, :],
                start=(ec == 0), stop=(ec == EC - 1))
        if half == 1:
            nc.vector.tensor_copy(
                out=v_t[:, kc, :, 0:D],
                in_=ps[:, 0:DC].rearrange("p (h d) -> p h d", h=GH))

    def attention_scores(qc, hc, kc):
        # Head pair (2*hc, 2*hc+1): head hp=0 on SBUF partitions 0-63, hp=1
        # on 64-127, so the two scores matmuls run as independent 64x128 PE
        # tiles and one ACTIVATE covers both heads' exp.
        sco = ps_sco.tile([128, 2, 512], F32, tag="sco", name="sco")
        for hp in range(2):
            po = hp * 64
            nc.tensor.matmul(
                sco[:, hp, :],
                lhsT=kT_t[po:po + 64, hc, kc * 128:(kc + 1) * 128],
                rhs=qT_t[po:po + 64, hc, qc * 512:(qc + 1) * 512],
                start=True, stop=True)
        pT = sb_p.tile([128, 2, 512], MM_DT)
        nc.scalar.activation(
            out=pT[:, :, :], in_=sco[:, :, :], func=EXP_FUNC,
            scale=float(SCALE))
        return pT

    def pv_alloc():
        return [ps_acc.tile([128, 512], F32, tag="acc", name=f"acc{j}")
                for j in range(2)]

    def pv_kc(accs, hc, pTs, kc):
        for hp in range(2):
            h = 2 * hc + hp
            nc.tensor.matmul(
                accs[hp][0:D + 1, :],
                lhsT=v_t[:, kc, h, :],
                rhs=pTs[kc][:, hp, :],
                start=(kc == 0), stop=(kc == KC - 1))

    def attention_norm(qc, hc, accs):
        for hp in range(2):
            po = hp * 64
            rs = sb_norm.tile([1, 512], F32, tag="rs")
            nc.vector.tensor_copy(out=rs[:, :], in_=accs[hp][D:D + 1, :])
            inv_r = sb_norm.tile([1, 512], F32, tag="inv")
            nc.vector.reciprocal_approx_fast(out=inv_r[:, :], in_=rs[:, :])
            brd = sb_norm.tile([64, 512], F32, tag="brd")
            nc.gpsimd.partition_broadcast(brd[:, :], inv_r[:, :])
            nc.vector.tensor_mul(
                o_t[po:po + 64, hc, qc * 512:(qc + 1) * 512],
                accs[hp][0:D, :],
                brd[:, :])

    def y_group(qc, ec, copy_eng=None):
        # psum[e, s] += Wo[c, e].T @ O^T[c, s] for chunk (ec, qc)
        yp = ps_acc.tile([128, 512], F32, tag="acc", name="yp")
        for cc in range(2):
            nc.tensor.matmul(
                yp[:, :],
                lhsT=wo_t[:, cc, ec * 128:(ec + 1) * 128],
                rhs=o_t[:, cc, qc * 512:(qc + 1) * 512],
                start=(cc == 0), stop=(cc == 1))
        ys = sb_y.tile([128, 512], F32)
        if copy_eng == "scalar":
            nc.scalar.copy(out=ys[:, :], in_=yp[:, :])
        else:
            nc.vector.tensor_copy(out=ys[:, :], in_=yp[:, :])
        nc.sync.dma_start(
            out=yT[ec * 128:(ec + 1) * 128, qc * 512:(qc + 1) * 512],
            in_=ys[:, :])

    # --- software-pipelined emission (= Tile priority order) ------------
    # Per kc slot: next scores pair (feeds ACT), previous block's PV, and
    # fillers drained at ~1/slot so PE tracks just under the ACT rate.
    blocks = [(0, 0), (1, 0), (2, 0), (3, 0), (0, 1), (1, 1), (2, 1), (3, 1)]

    def qk(dc, proj, sc, half):
        return lambda: qk_part(dc, proj, sc, half)

    def vp(kc, half):
        return lambda: v_part(kc, half)

    def yg(qc, ec):
        return lambda: y_group(qc, ec)

    # filler lists per block index; deadlines:
    #   K(dc0,sc) before block 0 reaches kc=4*sc; Q(dc0,sc) before block sc;
    #   K(dc1,*)+Q(dc1,0) before block 4; Q(dc1,sc) before block 4+sc;
    #   v(kc) emitted before the PV stream (block 1) reaches kc — REQUIRED,
    #   the 4-slot PSUM pool can otherwise hit a circular slot wait;
    #   y(qc) after norm(qc,1) (block 5+qc).
    fillers = {
        0: [qk(0, 1, 1, 0), qk(0, 1, 1, 1), qk(0, 0, 1, 0),
            qk(0, 1, 2, 0), qk(0, 1, 2, 1), qk(0, 0, 1, 1),
            vp(0, 0), vp(0, 1),
            qk(0, 1, 3, 0), qk(0, 1, 3, 1),
            vp(1, 0), vp(1, 1), vp(2, 0), vp(2, 1),
            vp(3, 0), vp(3, 1), vp(4, 0), vp(4, 1),
            vp(5, 0), vp(5, 1), vp(6, 0), vp(6, 1),
            vp(7, 0), vp(7, 1), vp(8, 0), vp(8, 1)],
        1: [qk(0, 0, 2, 0), qk(0, 0, 2, 1),
            vp(9, 0), vp(9, 1),
            vp(10, 0), vp(10, 1), vp(11, 0), vp(11, 1),
            vp(12, 0), vp(12, 1), vp(13, 0), vp(13, 1),
            vp(14, 0), vp(14, 1), vp(15, 0), vp(15, 1)],
        2: [qk(0, 0, 3, 0), qk(0, 0, 3, 1),
            qk(1, 1, 0, 0), qk(1, 1, 0, 1), qk(1, 1, 1, 0), qk(1, 1, 1, 1)],
        3: [qk(1, 1, 2, 0), qk(1, 1, 2, 1), qk(1, 1, 3, 0), qk(1, 1, 3, 1),
            qk(1, 0, 0, 0), qk(1, 0, 0, 1)],
        4: [qk(1, 0, 1, 0), qk(1, 0, 1, 1)],
        5: [qk(1, 0, 2, 0), qk(1, 0, 2, 1)],
        6: [qk(1, 0, 3, 0), qk(1, 0, 3, 1)] + [yg(0, e) for e in range(EC)],
        7: [yg(1, e) for e in range(EC)],
    }

    # block 0: projections for its own first tiles, then the kc stream
    qk_part(0, 0, 0, 0)
    qk_part(0, 0, 0, 1)
    qk_part(0, 1, 0, 0)
    qk_part(0, 1, 0, 1)

    pts_prev = None
    prev_block = None
    for bi, (qc, hc) in enumerate(blocks):
        accs = pv_alloc() if bi > 0 else None
        fl = list(fillers.get(bi, []))
        ndrain = 0
        pts_cur = []
        for kc in range(KC):
            pts_cur.append(attention_scores(qc, hc, kc))
            want = (kc + 1) * len(fl) // KC
            while ndrain < want:
                fl[ndrain]()
                ndrain += 1
            if bi > 0:
                pv_kc(accs, prev_block[1], pts_prev, kc)
        if bi > 0:
            attention_norm(prev_block[0], prev_block[1], accs)
        pts_prev = pts_cur
        prev_block = (qc, hc)
    # final block: its own PV (2-kc lag) + norm + remaining y chunks
    accs = pv_alloc()
    for kc in range(KC):
        pv_kc(accs, prev_block[1], pts_prev, kc)
        if kc % 2 == 1:
            y_group(2, kc // 2)
    attention_norm(prev_block[0], prev_block[1], accs)
    for ec in range(EC):
        y_group(3, ec, copy_eng="scalar" if ec % 2 else None)

    if dbg is not None:
        for name, t in (("qT", qT_t), ("kT", kT_t), ("o", o_t)):
            f = sb_big.tile([128, 2, S], F32, name=f"dump_{name}")
            nc.vector.tensor_copy(out=f[:, :, :], in_=t[:, :, :])
            nc.sync.dma_start(out=dbg[name], in_=f.rearrange("p a b -> p (a b)"))
        fv = sb_big.tile([128, KC, GH, D + 1], F32, name="dump_v")
        nc.vector.tensor_copy(out=fv[:, :, :, :], in_=v_t[:, :, :, :])
        nc.sync.dma_start(out=dbg["v"], in_=fv.rearrange("p a b c -> p (a b c)"))


_cached_nc = None


def _build():
    nc = bacc.Bacc(trn_type="TRN2", target_bir_lowering=False)
    xT = nc.dram_tensor("xT", [128, EC * S], MM_DT, kind="ExternalInput").ap()
    wq = nc.dram_tensor("wq", [128, EC * DC], MM_DT, kind="ExternalInput").ap()
    wk = nc.dram_tensor("wk", [128, EC * DC], MM_DT, kind="ExternalInput").ap()
    wv = nc.dram_tensor("wv", [128, EC * DC], MM_DT, kind="ExternalInput").ap()
    wo = nc.dram_tensor("wo", [128, 2 * E], MM_DT, kind="ExternalInput").ap()
    bqk = nc.dram_tensor("bqk", [128, 4], F32, kind="ExternalInput").ap()
    yT = nc.dram_tensor("yT", [E, S], F32, kind="ExternalOutput").ap()
    dbg = None
    if DEBUG_DUMPS:
        dbg = {
            "qT": nc.dram_tensor("dbg_qT", [128, 2 * S], F32, kind="ExternalOutput").ap(),
            "kT": nc.dram_tensor("dbg_kT", [128, 2 * S], F32, kind="ExternalOutput").ap(),
            "o": nc.dram_tensor("dbg_o", [128, 2 * S], F32, kind="ExternalOutput").ap(),
            "v": nc.dram_tensor("dbg_v", [128, KC * GH * (D + 1)], F32, kind="ExternalOutput").ap(),
        }
    with tile.TileContext(nc) as tc:
        with ExitStack() as ctx:
            _emit(nc, tc, ctx, xT, wq, wk, wv, wo, bqk, yT, dbg)
    nc.compile()
    return nc


def get_nc():
    global _cached_nc
    if _cached_nc is None:
        _cached_nc = _build()
    return _cached_nc


def make_in_maps(inputs, wq, bq, wk, bk, wv, wo):
    in_maps = []
    for c in range(NCORES):
        b, g = divmod(c, GH)
        sl = slice(g * DC, (g + 1) * DC)
        def perm(a):
            # [C*128, N] -> [128, C*N] with SBUF chunk-major free dim
            cN = a.shape[0] // 128
            return np.ascontiguousarray(
                a.reshape(cN, 128, a.shape[1]).transpose(1, 0, 2).reshape(
                    128, cN * a.shape[1]))

        bq_g, bk_g = bq[sl], bk[sl]
        bqk = np.stack([bq_g[0:128], bq_g[128:256],
                        bk_g[0:128], bk_g[128:256]], axis=1)
        in_maps.append({
            "xT": round_f32r(perm(np.ascontiguousarray(inputs[b].T))),
            "wq": round_f32r(perm(wq[:, sl])),
            "wk": round_f32r(perm(wk[:, sl])),
            "wv": round_f32r(perm(wv[:, sl])),
            "wo": round_f32r(perm(wo[sl, :])),
            "bqk": np.ascontiguousarray(bqk, np.float32),
        })
    return in_maps


def combine(results, wv_full, bv, wo_full, bo):
    y = np.zeros((B, S, E), np.float32)
    for c in range(NCORES):
        y[c // GH] += results[c]["yT"].T
    y += bv @ wo_full + bo
    return y


def kernel(inputs, wq, bq, wk, bk, wv, bv, wo, bo, _run_kwargs=None):
    inputs = np.asarray(inputs, np.float32)
    wq, bq = np.asarray(wq, np.float32), np.asarray(bq, np.float32)
    wk, bk = np.asarray(wk, np.float32), np.asarray(bk, np.float32)
    wv, bv = np.asarray(wv, np.float32), np.asarray(bv, np.float32)
    wo, bo = np.asarray(wo, np.float32), np.asarray(bo, np.float32)

    nc = get_nc()
    in_maps = make_in_maps(inputs, wq, bq, wk, bk, wv, wo)
    res = run_bass_kernel_spmd(nc, in_maps, list(range(NCORES)),
                               **(_run_kwargs or {}))
    y = combine(res.results, wv, bv, wo, bo)
    if _run_kwargs:
        kernel.last_result = res
    return y


# revision 10
# speedup vs baseline: 1.0726x; 1.0130x over previous
"""Multi-head self-attention (B=2, S=2048, E=1024, H=16, D=64) on 8 NeuronCores.

Sharding: core c -> (batch b = c // 4, head group g = c % 4).  Each core
computes Q/K/V projections for its 4 heads (column-parallel), attention, and
a partial output projection (row-parallel); the host sums the 4 partials per
batch.  All device activations live in "transposed space" (feature on the
partition dim) so every matmul contracts along partitions with no on-device
transposes:

  Q^T = Wq_g^T @ X^T          [256, 2048]  (e-chunk accumulated; bias via DVE)
  K^T = Wk_g^T @ X^T          [256, 2048]
  V   = X @ Wv_g              [2048, 256]  (natural; ones column appended)
  S^T = K_h @ Q_h^T / 8       [2048, 2048] per head (computed tile-wise)
  P^T = exp(S^T)              (softmax without max-subtraction: scores ~N(0,1))
  O'^T = [V_h | 1]^T @ P^T    [65, q]  (row 64 = softmax denominators)
  O^T  = O'[0:64] / O'[64]    (DVE reciprocal + GpSimd partition broadcast)
  Y^T  = Wo_g^T @ O^T         [1024, 2048] partial, host-summed per batch

bv and bo are folded on the host (exact: softmax rows sum to 1, so
attn(V + bv) = attn(V) + bv, and the output projection is linear).

Schedule: the exp stream (ScalarE) and the matmul stream (PE) are both near
their engine floors (~128us and ~140us), so the emission order software-
pipelines them: per k-chunk the PE emits the next scores pair + the previous
block's PV accumulation + one "filler" (projection / output chunk) sized to
keep PE just under the ACT rate.  Warmup matmuls + a dummy exp run during the
initial DMA so the PE starts HAM-warm and the exp table set is preloaded.
"""

from contextlib import ExitStack

import numpy as np

import concourse.bass as bass
import concourse.tile as tile
from concourse import bacc, mybir
from concourse.bass_utils import run_bass_kernel_spmd

B, S, E, H, D = 2, 2048, 1024, 16, 64
NCORES = 8
GH = 4            # heads per core
DC = GH * D       # head-dim columns per core (256)
EC = E // 128     # 8 e-chunks
KC = S // 128     # 16 k-chunks
F32 = mybir.dt.float32
MM_DT = mybir.dt.float16    # full-speed 16-bit matmul path (10-bit mantissa)
EXP_FUNC = mybir.ActivationFunctionType.Exp
ADD = mybir.AluOpType.add
SCALE = 1.0 / np.sqrt(np.float32(D))


def round_f32r(a):
    # Host-side conversion to the matmul dtype (RNE)
    if MM_DT == mybir.dt.float16:
        return np.ascontiguousarray(a, np.float32).astype(np.float16)
    if MM_DT == mybir.dt.bfloat16:
        import ml_dtypes
        return np.ascontiguousarray(a, np.float32).astype(ml_dtypes.bfloat16)
    return np.ascontiguousarray(a, np.float32)


DEBUG_DUMPS = False


def _emit(nc, tc, ctx, xT, wq, wk, wv, wo, bqk, yT, dbg=None):
    sb_big = ctx.enter_context(tc.tile_pool(name="sb_big", bufs=1))
    sb_p = ctx.enter_context(tc.tile_pool(name="sb_p", bufs=28))
    sb_norm = ctx.enter_context(tc.tile_pool(name="sb_norm", bufs=4))
    sb_y = ctx.enter_context(tc.tile_pool(name="sb_y", bufs=2))
    ps_sco = ctx.enter_context(tc.tile_pool(name="ps_sco", bufs=2, space="PSUM"))
    ps_acc = ctx.enter_context(tc.tile_pool(name="ps_acc", bufs=4, space="PSUM"))

    xT_t = sb_big.tile([128, EC, S], MM_DT)
    wq_t = sb_big.tile([128, EC, DC], MM_DT)
    wk_t = sb_big.tile([128, EC, DC], MM_DT)
    wv_t = sb_big.tile([128, EC, DC], MM_DT)
    wo_t = sb_big.tile([128, 2, E], MM_DT)
    bqk_t = sb_big.tile([128, 4], F32)
    qT_t = sb_big.tile([128, 2, S], MM_DT)
    kT_t = sb_big.tile([128, 2, S], MM_DT)
    v_t = sb_big.tile([128, KC, GH, D + 1], MM_DT)
    o_t = sb_big.tile([128, 2, S], MM_DT)
    warm_t = sb_big.tile([128, 512], MM_DT)
    warm_o = sb_big.tile([128, 512], MM_DT)

    # --- warmup: PE busy + exp table preload while input DMAs run -------
    nc.vector.memset(warm_t[:, :], 0.125)
    nc.scalar.activation(out=warm_o[:, :], in_=warm_t[:, :], func=EXP_FUNC,
                         scale=float(SCALE))
    warm_ps = ps_acc.tile([128, 512], F32, tag="acc", name="warm")
    for i in range(6):
        nc.tensor.matmul(warm_ps[:, :], lhsT=warm_t[:, 0:128],
                         rhs=warm_t[:, :], start=True, stop=True)
    for kc in range(KC):
        nc.vector.memset(v_t[:, kc, :, D:D + 1], 1.0)

    # --- input DMAs: ordered so first-needed tiles land first -----------
    # sync queue: wq(dc0), x halves (ec 0-3) per sc, wv, wo(c0)
    # gpsimd queue: bias, wk(dc0), x halves (ec 4-7) per sc, wq/wk(dc1), wo(c1)
    wq_r = wq.rearrange("p (c d) -> p c d", c=EC)
    wk_r = wk.rearrange("p (c d) -> p c d", c=EC)
    x_r = xT.rearrange("p (c s) -> p c s", c=EC)
    wo_r = wo.rearrange("p (c e) -> p c e", c=2)
    nc.sync.dma_start(out=wq_t[:, :, 0:128], in_=wq_r[:, :, 0:128])
    nc.gpsimd.dma_start(out=bqk_t[:, :], in_=bqk)
    nc.gpsimd.dma_start(out=wk_t[:, :, 0:128], in_=wk_r[:, :, 0:128])
    for sc in range(4):
        sl = slice(sc * 512, (sc + 1) * 512)
        nc.sync.dma_start(out=xT_t[:, 0:4, sl], in_=x_r[:, 0:4, sl])
        nc.gpsimd.dma_start(out=xT_t[:, 4:8, sl], in_=x_r[:, 4:8, sl])
    nc.sync.dma_start(out=wv_t[:, :, :], in_=wv.rearrange(
        "p (c d) -> p c d", c=EC))
    nc.gpsimd.dma_start(out=wq_t[:, :, 128:256], in_=wq_r[:, :, 128:256])
    nc.gpsimd.dma_start(out=wk_t[:, :, 128:256], in_=wk_r[:, :, 128:256])
    nc.sync.dma_start(out=wo_t[:, 0, :], in_=wo_r[:, 0, :])
    nc.gpsimd.dma_start(out=wo_t[:, 1, :], in_=wo_r[:, 1, :])

    def qk_part(dc, proj, sc, half, state={}):
        # psum[d, s] += W[e, d].T @ X^T[e, s], two halves so bursts stay
        # small; bias folded into the DVE evacuation copy.
        w_t, dst = ((wq_t, qT_t), (wk_t, kT_t))[proj]
        if half == 0:
            state[(dc, proj, sc)] = ps_acc.tile(
                [128, 512], F32, tag="acc", name="ps_qk")
        ps = state[(dc, proj, sc)]
        ecs = range(EC // 2) if half == 0 else range(EC // 2, EC)
        for ec in ecs:
            nc.tensor.matmul(
                ps[:, :],
                lhsT=w_t[:, ec, dc * 128:(dc + 1) * 128],
                rhs=xT_t[:, ec, sc * 512:(sc + 1) * 512],
                start=(ec == 0), stop=(ec == EC - 1))
        if half == 1:
            nc.vector.tensor_scalar(
                out=dst[:, dc, sc * 512:(sc + 1) * 512], in0=ps[:, :],
                scalar1=bqk_t[:, 2 * proj + dc:2 * proj + dc + 1],
                scalar2=None, op0=ADD)
            del state[(dc, proj, sc)]

    def v_part(kc, half):
        # psum[s, d] += X^T[e, s].T @ Wv[e, d]
        if half == 0:
            v_part.ps = ps_acc.tile([128, 512], F32, tag="acc", name="ps_v")
        ps = v_part.ps
        ecs = range(EC // 2) if half == 0 else range(EC // 2, EC)
        for ec in ecs:
            nc.tensor.matmul(
                ps[:, 0:DC],
                lhsT=xT_t[:, ec, kc * 128:(kc + 1) * 128],
                rhs=wv_t[:, ec, :],
                start=(ec == 0), stop=(ec == EC - 1))
        if half == 1:
            nc.vector.tensor_copy(
                out=v_t[:, kc, :, 0:D],
                in_=ps[:, 0:DC].rearrange("p (h d) -> p h d", h=GH))

    def attention_scores(qc, hc, kc):
        # Head pair (2*hc, 2*hc+1): head hp=0 on SBUF partitions 0-63, hp=1
        # on 64-127, so the two scores matmuls run as independent 64x128 PE
        # tiles and one ACTIVATE covers both heads' exp.
        sco = ps_sco.tile([128, 2, 512], F32, tag="sco", name="sco")
        for hp in range(2):
            po = hp * 64
            nc.tensor.matmul(
                sco[:, hp, :],
                lhsT=kT_t[po:po + 64, hc, kc * 128:(kc + 1) * 128],
                rhs=qT_t[po:po + 64, hc, qc * 512:(qc + 1) * 512],
                start=True, stop=True)
        pT = sb_p.tile([128, 2, 512], MM_DT)
        nc.scalar.activation(
            out=pT[:, :, :], in_=sco[:, :, :], func=EXP_FUNC,
            scale=float(SCALE))
        return pT

    def pv_alloc():
        return [ps_acc.tile([128, 512], F32, tag="acc", name=f"acc{j}")
                for j in range(2)]

    def pv_kc(accs, hc, pTs, kc):
        for hp in range(2):
            h = 2 * hc + hp
            nc.tensor.matmul(
                accs[hp][0:D + 1, :],
                lhsT=v_t[:, kc, h, :],
                rhs=pTs[kc][:, hp, :],
                start=(kc == 0), stop=(kc == KC - 1))

    def attention_norm(qc, hc, accs):
        for hp in range(2):
            po = hp * 64
            rs = sb_norm.tile([1, 512], F32, tag="rs")
            nc.vector.tensor_copy(out=rs[:, :], in_=accs[hp][D:D + 1, :])
            inv_r = sb_norm.tile([1, 512], F32, tag="inv")
            nc.vector.reciprocal_approx_fast(out=inv_r[:, :], in_=rs[:, :])
            brd = sb_norm.tile([64, 512], F32, tag="brd")
            nc.gpsimd.partition_broadcast(brd[:, :], inv_r[:, :])
            nc.vector.tensor_mul(
                o_t[po:po + 64, hc, qc * 512:(qc + 1) * 512],
                accs[hp][0:D, :],
                brd[:, :])

    def y_group(qc, ec, copy_eng=None):
        # psum[e, s] += Wo[c, e].T @ O^T[c, s] for chunk (ec, qc)
        yp = ps_acc.tile([128, 512], F32, tag="acc", name="yp")
        for cc in range(2):
            nc.tensor.matmul(
                yp[:, :],
                lhsT=wo_t[:, cc, ec * 128:(ec + 1) * 128],
                rhs=o_t[:, cc, qc * 512:(qc + 1) * 512],
                start=(cc == 0), stop=(cc == 1))
        ys = sb_y.tile([128, 512], F32)
        if copy_eng == "scalar":
            nc.scalar.copy(out=ys[:, :], in_=yp[:, :])
        else:
            nc.vector.tensor_copy(out=ys[:, :], in_=yp[:, :])
        nc.sync.dma_start(
            out=yT[ec * 128:(ec + 1) * 128, qc * 512:(qc + 1) * 512],
            in_=ys[:, :])

    # --- software-pipelined emission (= Tile priority order) ------------
    # Per kc slot: next scores pair (feeds ACT), previous block's PV, and
    # fillers drained at ~1/slot so PE tracks just under the ACT rate.
    # hc-inner order so each qc's second norm lands mid-schedule and the
    # y-projection chunks spread into blocks 4-7 instead of the tail.
    blocks = [(0, 0), (1, 0), (0, 1), (1, 1), (2, 0), (2, 1), (3, 0), (3, 1)]

    def qk(dc, proj, sc, half):
        return lambda: qk_part(dc, proj, sc, half)

    def vp(kc, half):
        return lambda: v_part(kc, half)

    def yg(qc, ec):
        return lambda: y_group(qc, ec)

    # filler lists per block index; deadlines:
    #   K(dc0,sc) before block 0 reaches kc=4*sc; Q(dc0,s1) before block 1;
    #   K(dc1,sc)+Q(dc1,s0) before block 2 reaches kc=4*sc;
    #   Q(dc1,s1) < blk3; Q(dc0,s2) < blk4; Q(dc1,s2) < blk5;
    #   Q(dc0,s3) < blk6; Q(dc1,s3) < blk7;
    #   v(kc) emitted before the PV stream (block 1) reaches kc — REQUIRED,
    #   the 4-slot PSUM pool can otherwise hit a circular slot wait;
    #   y(qc) after norm(qc,1): y0 > blk3, y1 > blk4, y2 > blk6, y3 tail.
    fillers = {
        0: [qk(0, 1, 1, 0), qk(0, 1, 1, 1), qk(0, 0, 1, 0),
            qk(0, 1, 2, 0), qk(0, 1, 2, 1), qk(0, 0, 1, 1),
            vp(0, 0), vp(0, 1),
            qk(0, 1, 3, 0), qk(0, 1, 3, 1),
            vp(1, 0), vp(1, 1), vp(2, 0), vp(2, 1),
            vp(3, 0), vp(3, 1), vp(4, 0), vp(4, 1),
            vp(5, 0), vp(5, 1), vp(6, 0), vp(6, 1),
            vp(7, 0), vp(7, 1), vp(8, 0), vp(8, 1),
            vp(9, 0), vp(9, 1), vp(10, 0), vp(10, 1),
            vp(11, 0), vp(11, 1)],
        1: [qk(1, 1, 0, 0), qk(1, 1, 0, 1), qk(1, 0, 0, 0), qk(1, 0, 0, 1),
            vp(12, 0), vp(12, 1), vp(13, 0), vp(13, 1),
            vp(14, 0), vp(14, 1), vp(15, 0), vp(15, 1)],
        2: [qk(1, 1, 1, 0), qk(1, 1, 1, 1), qk(1, 1, 2, 0), qk(1, 1, 2, 1),
            qk(1, 1, 3, 0), qk(1, 1, 3, 1), qk(1, 0, 1, 0), qk(1, 0, 1, 1)],
        3: [qk(0, 0, 2, 0), qk(0, 0, 2, 1)],
        4: [qk(1, 0, 2, 0), qk(1, 0, 2, 1)] + [yg(0, e) for e in range(EC)],
        5: [qk(0, 0, 3, 0), qk(0, 0, 3, 1)] + [yg(1, e) for e in range(EC)],
        6: [qk(1, 0, 3, 0), qk(1, 0, 3, 1)],
        7: [yg(2, e) for e in range(EC)],
    }

    # block 0: projections for its own first tiles, then the kc stream
    qk_part(0, 0, 0, 0)
    qk_part(0, 0, 0, 1)
    qk_part(0, 1, 0, 0)
    qk_part(0, 1, 0, 1)

    pts_prev = None
    prev_block = None
    for bi, (qc, hc) in enumerate(blocks):
        accs = pv_alloc() if bi > 0 else None
        fl = list(fillers.get(bi, []))
        ndrain = 0
        pts_cur = []
        for kc in range(0, KC, 2):
            # scores pairs back-to-back: the second pair's kT LDWEIGHTS
            # overlaps the first pair's matmuls (disjoint PE row groups)
            pts_cur.append(attention_scores(qc, hc, kc))
            pts_cur.append(attention_scores(qc, hc, kc + 1))
            want = (kc + 2) * len(fl) // KC
            while ndrain < want:
                fl[ndrain]()
                ndrain += 1
            if bi > 0:
                pv_kc(accs, prev_block[1], pts_prev, kc)
                pv_kc(accs, prev_block[1], pts_prev, kc + 1)
        if bi > 0:
            attention_norm(prev_block[0], prev_block[1], accs)
        pts_prev = pts_cur
        prev_block = (qc, hc)
    # final block: its own PV + norm + the last y chunks
    accs = pv_alloc()
    for kc in range(KC):
        pv_kc(accs, prev_block[1], pts_prev, kc)
    attention_norm(prev_block[0], prev_block[1], accs)
    for ec in range(EC):
        y_group(3, ec, copy_eng="scalar" if ec % 2 else None)

    if dbg is not None:
        for name, t in (("qT", qT_t), ("kT", kT_t), ("o", o_t)):
            f = sb_big.tile([128, 2, S], F32, name=f"dump_{name}")
            nc.vector.tensor_copy(out=f[:, :, :], in_=t[:, :, :])
            nc.sync.dma_start(out=dbg[name], in_=f.rearrange("p a b -> p (a b)"))
        fv = sb_big.tile([128, KC, GH, D + 1], F32, name="dump_v")
        nc.vector.tensor_copy(out=fv[:, :, :, :], in_=v_t[:, :, :, :])
        nc.sync.dma_start(out=dbg["v"], in_=fv.rearrange("p a b c -> p (a b c)"))


_cached_nc = None


def _build():
    nc = bacc.Bacc(trn_type="TRN2", target_bir_lowering=False)
    xT = nc.dram_tensor("xT", [128, EC * S], MM_DT, kind="ExternalInput").ap()
    wq = nc.dram_tensor("wq", [128, EC * DC], MM_DT, kind="ExternalInput").ap()
    wk = nc.dram_tensor("wk", [128, EC * DC], MM_DT, kind="ExternalInput").ap()
    wv = nc.dram_tensor("wv", [128, EC * DC], MM_DT, kind="ExternalInput").ap()
    wo = nc.dram_tensor("wo", [128, 2 * E], MM_DT, kind="ExternalInput").ap()
    bqk = nc.dram_tensor("bqk", [128, 4], F32, kind="ExternalInput").ap()
    yT = nc.dram_tensor("yT", [E, S], F32, kind="ExternalOutput").ap()
    dbg = None
    if DEBUG_DUMPS:
        dbg = {
            "qT": nc.dram_tensor("dbg_qT", [128, 2 * S], F32, kind="ExternalOutput").ap(),
            "kT": nc.dram_tensor("dbg_kT", [128, 2 * S], F32, kind="ExternalOutput").ap(),
            "o": nc.dram_tensor("dbg_o", [128, 2 * S], F32, kind="ExternalOutput").ap(),
            "v": nc.dram_tensor("dbg_v", [128, KC * GH * (D + 1)], F32, kind="ExternalOutput").ap(),
        }
    with tile.TileContext(nc) as tc:
        with ExitStack() as ctx:
            _emit(nc, tc, ctx, xT, wq, wk, wv, wo, bqk, yT, dbg)
    nc.compile()
    return nc


def get_nc():
    global _cached_nc
    if _cached_nc is None:
        _cached_nc = _build()
    return _cached_nc


def make_in_maps(inputs, wq, bq, wk, bk, wv, wo):
    in_maps = []
    for c in range(NCORES):
        b, g = divmod(c, GH)
        sl = slice(g * DC, (g + 1) * DC)
        def perm(a):
            # [C*128, N] -> [128, C*N] with SBUF chunk-major free dim
            cN = a.shape[0] // 128
            return np.ascontiguousarray(
                a.reshape(cN, 128, a.shape[1]).transpose(1, 0, 2).reshape(
                    128, cN * a.shape[1]))

        bq_g, bk_g = bq[sl], bk[sl]
        bqk = np.stack([bq_g[0:128], bq_g[128:256],
                        bk_g[0:128], bk_g[128:256]], axis=1)
        in_maps.append({
            "xT": round_f32r(perm(np.ascontiguousarray(inputs[b].T))),
            "wq": round_f32r(perm(wq[:, sl])),
            "wk": round_f32r(perm(wk[:, sl])),
            "wv": round_f32r(perm(wv[:, sl])),
            "wo": round_f32r(perm(wo[sl, :])),
            "bqk": np.ascontiguousarray(bqk, np.float32),
        })
    return in_maps


def combine(results, wv_full, bv, wo_full, bo):
    y = np.zeros((B, S, E), np.float32)
    for c in range(NCORES):
        y[c // GH] += results[c]["yT"].T
    y += bv @ wo_full + bo
    return y


def kernel(inputs, wq, bq, wk, bk, wv, bv, wo, bo, _run_kwargs=None):
    inputs = np.asarray(inputs, np.float32)
    wq, bq = np.asarray(wq, np.float32), np.asarray(bq, np.float32)
    wk, bk = np.asarray(wk, np.float32), np.asarray(bk, np.float32)
    wv, bv = np.asarray(wv, np.float32), np.asarray(bv, np.float32)
    wo, bo = np.asarray(wo, np.float32), np.asarray(bo, np.float32)

    nc = get_nc()
    in_maps = make_in_maps(inputs, wq, bq, wk, bk, wv, wo)
    res = run_bass_kernel_spmd(nc, in_maps, list(range(NCORES)),
                               **(_run_kwargs or {}))
    y = combine(res.results, wv, bv, wo, bo)
    if _run_kwargs:
        kernel.last_result = res
    return y
